# revision 15
# baseline (speedup 1.0000x reference)
"""MoE transformer block (QK-norm attention + top-8-of-16 MoE) on 8 trn2 cores.

Sharding: attention head-parallel (core c owns head c, both batches),
experts expert-parallel (core c owns experts 2c, 2c+1). Dense expert eval
(gates zero out unselected tokens -> matches the reference math exactly).
Two AllReduces: attention-proj partials, MoE partials.

Everything runs in "T layout" (feature dim on partitions, tokens on free) so
matmul contractions are over partitions. QK-normalized scores are bounded
(|s| <= alpha), so softmax skips max-subtraction; the denominator is a
ones-matmul over the k partition axis -> no transposes inside attention.
"""

import numpy as np
import ml_dtypes

import concourse.bass as bass
import concourse.mybir as mybir
from concourse.tile import TileContext
from concourse.masks import make_identity
from concourse.bass_utils import run_bass_kernel_spmd

BF16 = mybir.dt.bfloat16
F32 = mybir.dt.float32
AFT = mybir.ActivationFunctionType
MUL = mybir.AluOpType.mult
ADD = mybir.AluOpType.add

P = 128
D = 512          # embed dim
T = 1024         # tokens per batch
N = 2048         # total tokens
E = 16           # experts
EL = 2           # experts per core
HD = 2048        # expert hidden
HDIM = 64        # head dim
NB = 4           # expert-phase token blocks
TB = N // NB     # 512 tokens per block
NCORES = 8

_cache = {}


def build_program():
    nc = bass.Bass()
    dp_ = dict(isOutput=False)
    x_d = nc.declare_dram_parameter("x", [N, D], F32, **dp_)
    gvec_d = nc.declare_dram_parameter("gvec", [4, P], F32, **dp_)
    bvec_d = nc.declare_dram_parameter("bvec", [4, P], F32, **dp_)
    wqkv_d = nc.declare_dram_parameter("wqkv", [D, 192], BF16, **dp_)
    bqkv_d = nc.declare_dram_parameter("bqkv", [3, HDIM], F32, **dp_)
    alpha_d = nc.declare_dram_parameter("alpha_s", [1, 1], F32, **dp_)
    maskt_d = nc.declare_dram_parameter("maskt", [T, T], BF16, **dp_)
    wproj_d = nc.declare_dram_parameter("wproj", [HDIM, D], BF16, **dp_)
    projb_d = nc.declare_dram_parameter("projb_bc", [P, D], F32, **dp_)
    projv_d = nc.declare_dram_parameter("projb_vec", [4, P], F32, **dp_)
    vbias_d = nc.declare_dram_parameter("vbias_bc", [P, HDIM], F32, **dp_)
    rw_d = nc.declare_dram_parameter("rw", [D, E], BF16, **dp_)
    rb_d = nc.declare_dram_parameter("rb_bc", [P, E], F32, **dp_)
    sel_d = nc.declare_dram_parameter("selb", [EL, E, P], BF16, **dp_)
    win_d = nc.declare_dram_parameter("w_in_l", [EL, D, HD], BF16, **dp_)
    bin_d = nc.declare_dram_parameter("b_in_l", [EL, 16, P], F32, **dp_)
    w1_d = nc.declare_dram_parameter("w1_l", [EL, HD, 2 * HD], BF16, **dp_)
    b1_d = nc.declare_dram_parameter("b1_l", [EL, 32, P], F32, **dp_)
    w2_d = nc.declare_dram_parameter("w2_l", [EL, HD, HD], BF16, **dp_)
    b2_d = nc.declare_dram_parameter("b2_l", [EL, 16, P], F32, **dp_)
    wout_d = nc.declare_dram_parameter("w_out_l", [EL, HD, D], BF16, **dp_)
    bout_d = nc.declare_dram_parameter("b_out_l", [EL, 4, P], F32, **dp_)
    out_d = nc.declare_dram_parameter("out", [N, D], F32, isOutput=True)

    groups = [list(range(NCORES))]

    with TileContext(nc, num_cores=NCORES) as tc:
        with (
            tc.tile_pool(name="const", bufs=1) as cp,
            tc.tile_pool(name="pp", bufs=4) as pp,
            tc.tile_pool(name="psA", bufs=4, space="PSUM") as psA,
            tc.tile_pool(name="psB", bufs=2, space="PSUM") as psB,
            tc.tile_pool(name="psC", bufs=2, space="PSUM") as psC,
            tc.tile_pool(name="dram", bufs=1, space="DRAM") as dp,
        ):
            # ---- constants / small params (persist) ----
            ident = cp.tile([P, P], F32, tag="ident")
            make_identity(nc, ident)
            ones64 = cp.tile([HDIM, 1], F32, tag="ones64")
            nc.vector.memset(ones64, 1.0)
            ones128 = cp.tile([P, 1], F32, tag="ones128")
            nc.vector.memset(ones128, 1.0)
            ones1r = cp.tile([1, P], F32, tag="ones1r")
            nc.vector.memset(ones1r, 1.0)
            ones128b = cp.tile([P, 1], BF16, tag="ones128b")
            nc.vector.memset(ones128b, 1.0)
            g_sb = cp.tile([P, 4], F32, tag="g_sb")
            nc.sync.dma_start(g_sb, gvec_d[:, :].rearrange("c p -> p c"))
            b_sb = cp.tile([P, 4], F32, tag="b_sb")
            nc.sync.dma_start(b_sb, bvec_d[:, :].rearrange("c p -> p c"))
            sel_sb = cp.tile([E, EL, P], BF16, tag="sel_sb")
            nc.sync.dma_start(sel_sb, sel_d[:, :, :].rearrange("e k p -> k e p"))
            bin_sb = cp.tile([P, EL, 16], F32, tag="bin_sb")
            nc.sync.dma_start(bin_sb, bin_d[:, :, :].rearrange("e c p -> p e c"))
            b1_sb = cp.tile([P, EL, 32], F32, tag="b1_sb")
            nc.sync.dma_start(b1_sb, b1_d[:, :, :].rearrange("e c p -> p e c"))
            b2_sb = cp.tile([P, EL, 16], F32, tag="b2_sb")
            nc.sync.dma_start(b2_sb, b2_d[:, :, :].rearrange("e c p -> p e c"))
            bout_sb = cp.tile([P, EL, 4], F32, tag="bout_sb")
            nc.sync.dma_start(bout_sb, bout_d[:, :, :].rearrange("e c p -> p e c"))
            rw_sb = cp.tile([P, 4, E], BF16, tag="rw_sb")
            nc.sync.dma_start(rw_sb, rw_d[:, :].rearrange("(kc p) e -> p kc e", p=P))
            rb_sb = cp.tile([P, E], F32, tag="rb_sb")
            nc.sync.dma_start(rb_sb, rb_d[:, :])
            projv_sb = cp.tile([P, 4], F32, tag="projv_sb")
            nc.sync.dma_start(projv_sb, projv_d[:, :].rearrange("c p -> p c"))

            # ---- persistent activations ----
            xrTb = [pp.tile([P, N], BF16, tag="xrTb", name=f"xrTb{i}") for i in range(4)]
            moeT = [pp.tile([P, N], BF16, tag="moeT", name=f"moeT{i}") for i in range(4)]
            gatesT = pp.tile([E, N], BF16, tag="gatesT", bufs=1)
            gdram = dp.tile([N, E], BF16)
            xres_dram = dp.tile([N, D], F32)
            ar1_in = dp.tile([N, D], BF16)
            ar1_out = dp.tile([N, D], BF16, addr_space="Shared")
            ar2_in = dp.tile([4 * P, N], BF16)
            ar2_out = dp.tile([4 * P, N], BF16, addr_space="Shared")

            with tc.tile_pool(name="s1", bufs=4) as s1:
                xtm = s1.tile([P, 16, D], F32, tag="xtm", bufs=1)
                nc.sync.dma_start(xtm, x_d[:, :].rearrange("(g p) d -> p g d", p=P))
                xnT = [s1.tile([P, N], BF16, tag="xnT", name=f"xnT{i}") for i in range(4)]

                with tc.tile_pool(name="s1a", bufs=4) as s1a:
                    # PE primers: absorb const-memset and x-DMA waits so the
                    # transpose matmuls below carry at most one sync wait
                    # (walrus limit on LDW sync slots).
                    pprim = psC.tile([1, 1], F32, tag="ps_small")
                    nc.tensor.matmul(pprim, ident[:, 0:1], ident[:, 0:1],
                                     start=True, stop=True)
                    pprim2 = psC.tile([1, 1], F32, tag="ps_small")
                    nc.tensor.matmul(pprim2, xtm[:, 0, 0:1], xtm[:, 0, 0:1],
                                     start=True, stop=True)
                    pprim3 = psC.tile([1, 1], F32, tag="ps_small")
                    nc.tensor.matmul(pprim3, ones128, ones128,
                                     start=True, stop=True)
                    # transpose x -> xT
                    xT = [s1a.tile([P, N], F32, tag="xT", name=f"xT{i}") for i in range(4)]
                    for dc in range(4):
                        for g in range(16):
                            pt = psB.tile([P, P], F32, tag="tr")
                            nc.tensor.transpose(pt, xtm[:, g, dc * P:(dc + 1) * P], ident)
                            nc.scalar.activation(xT[dc][:, g * P:(g + 1) * P], pt,
                                                 AFT.Copy)
                    # rrow = 1/sqrt(mean(x^2) + 1e-6) as [1, N]
                    rrow = s1a.tile([1, N], F32, tag="rrow", bufs=1)
                    for nc4 in range(4):
                        sl = slice(nc4 * D, (nc4 + 1) * D)
                        ps = psC.tile([1, D], F32, tag="ps_small")
                        for dc in range(4):
                            sq = s1a.tile([P, D], F32, tag="sq_t", bufs=3)
                            nc.scalar.activation(sq, xT[dc][:, sl], AFT.Square)
                            nc.tensor.matmul(ps, ones128, sq,
                                             start=(dc == 0), stop=(dc == 3))
                        tmp = s1a.tile([1, D], F32, tag="r_t", bufs=2)
                        nc.vector.tensor_scalar(tmp, ps, 1.0 / D, 1e-6,
                                                op0=MUL, op1=ADD)
                        nc.scalar.activation(tmp, tmp, AFT.Sqrt)
                        nc.vector.reciprocal(rrow[0:1, sl], tmp)
                    # xnT = xT * bcast(rrow) * g + b   (bf16)
                    for nc4 in range(4):
                        sl = slice(nc4 * D, (nc4 + 1) * D)
                        pb = psB.tile([P, D], F32, tag="tr")
                        nc.tensor.matmul(pb, ones1r, rrow[0:1, sl],
                                         start=True, stop=True)
                        rb_bc = s1a.tile([P, D], F32, tag="rb_bc", bufs=2)
                        nc.scalar.activation(rb_bc, pb, AFT.Copy)
                        for dc in range(4):
                            t = s1a.tile([P, D], F32, tag="xn_t", bufs=3)
                            nc.vector.tensor_mul(t, xT[dc][:, sl], rb_bc)
                            nc.vector.tensor_scalar(
                                xnT[dc][:, sl], t,
                                g_sb[:, dc:dc + 1], b_sb[:, dc:dc + 1],
                                op0=MUL, op1=ADD)

                # ---- attention (own head, both batches) ----
                with tc.tile_pool(name="att", bufs=2) as at, \
                     tc.tile_pool(name="atte", bufs=12) as ate:
                    wq_sb = at.tile([P, 4, 192], BF16, tag="wq_sb", bufs=1)
                    nc.sync.dma_start(wq_sb,
                                      wqkv_d[:, :].rearrange("(kc p) m -> p kc m", p=P))
                    bq_sb = at.tile([HDIM, 3], F32, tag="bq_sb", bufs=1)
                    nc.sync.dma_start(bq_sb, bqkv_d[:, :].rearrange("i h -> h i"))
                    alpha_sb = at.tile([1, 1], F32, tag="alpha_sb", bufs=1)
                    nc.sync.dma_start(alpha_sb, alpha_d[:, :])
                    maskt_sb = at.tile([P, 8, T], BF16, tag="maskt_sb", bufs=1)
                    nc.sync.dma_start(maskt_sb,
                                      maskt_d[:, :].rearrange("(kc p) q -> p kc q", p=P))
                    wproj_sb = at.tile([HDIM, D], BF16, tag="wproj_sb", bufs=1)
                    nc.sync.dma_start(wproj_sb, wproj_d[:, :])
                    vbias_sb = at.tile([P, HDIM], F32, tag="vbias_sb", bufs=1)
                    nc.sync.dma_start(vbias_sb, vbias_d[:, :])

                    qT = at.tile([HDIM, N], F32, tag="qT", bufs=1)
                    kT = at.tile([HDIM, N], F32, tag="kT", bufs=1)
                    for wi, dst, bi in ((0, qT, 0), (1, kT, 1)):
                        for nc4 in range(4):
                            sl = slice(nc4 * D, (nc4 + 1) * D)
                            ps = psC.tile([HDIM, D], F32, tag="ps_small")
                            for kc in range(4):
                                nc.tensor.matmul(
                                    ps, wq_sb[:, kc, wi * HDIM:(wi + 1) * HDIM],
                                    xnT[kc][:, sl], start=(kc == 0), stop=(kc == 3))
                            nc.vector.tensor_scalar_add(dst[:, sl], ps,
                                                        bq_sb[:, bi:bi + 1])
                    # v token-major bf16
                    v_tm = at.tile([P, 16, HDIM], BF16, tag="v_tm", bufs=1)
                    for tk in range(16):
                        ps = psC.tile([P, HDIM], F32, tag="ps_small")
                        for kc in range(4):
                            nc.tensor.matmul(ps, xnT[kc][:, tk * P:(tk + 1) * P],
                                             wq_sb[:, kc, 128:192],
                                             start=(kc == 0), stop=(kc == 3))
                        tf = ate.tile([P, HDIM], F32, tag="v_ev", bufs=3)
                        nc.vector.tensor_add(tf, ps, vbias_sb)
                        nc.vector.tensor_copy(v_tm[:, tk, :], tf)
                    # q_hat (alpha folded) / k_hat
                    qh = at.tile([HDIM, N], BF16, tag="qh", bufs=1)
                    kh = at.tile([HDIM, N], BF16, tag="kh", bufs=1)
                    for src, dst, use_alpha in ((qT, qh, True), (kT, kh, False)):
                        rn = at.tile([1, N], F32, tag="rn", bufs=1)
                        for nc4 in range(4):
                            sl = slice(nc4 * D, (nc4 + 1) * D)
                            sq = ate.tile([HDIM, D], F32, tag="sqn", bufs=2)
                            nc.scalar.activation(sq, src[:, sl], AFT.Square)
                            ps = psC.tile([1, D], F32, tag="ps_small")
                            nc.tensor.matmul(ps, ones64, sq, start=True, stop=True)
                            t = ate.tile([1, D], F32, tag="rn_t", bufs=2)
                            nc.scalar.activation(t, ps, AFT.Sqrt)
                            nc.vector.tensor_scalar_add(t, t, 1e-5)
                            nc.vector.reciprocal(rn[0:1, sl], t)
                        if use_alpha:
                            nc.vector.tensor_scalar_mul(rn, rn, alpha_sb[0:1, 0:1])
                        for nc4 in range(4):
                            sl = slice(nc4 * D, (nc4 + 1) * D)
                            pb = psC.tile([HDIM, D], F32, tag="ps_small")
                            nc.tensor.matmul(pb, ones1r[0:1, 0:HDIM], rn[0:1, sl],
                                             start=True, stop=True)
                            nc.vector.tensor_mul(dst[:, sl], src[:, sl], pb)
                    # scoresT -> exp*mask -> denom + av
                    yhT = at.tile([HDIM, N], BF16, tag="yhT", bufs=1)
                    for b in range(2):
                        for qc in range(2):
                            qsl = slice(b * T + qc * D, b * T + (qc + 1) * D)
                            pd = psC.tile([1, D], F32, tag="ps_small")
                            py = psC.tile([HDIM, D], F32, tag="ps_small")
                            ex_tiles = []
                            for kc in range(8):
                                ksl = slice(b * T + kc * P, b * T + (kc + 1) * P)
                                ps = psA.tile([P, D], F32, tag="mm")
                                nc.tensor.matmul(ps, kh[:, ksl], qh[:, qsl],
                                                 start=True, stop=True)
                                et = ate.tile([P, D], BF16, tag="exp_b", bufs=8)
                                nc.scalar.activation(et, ps, AFT.Exp)
                                eb = ate.tile([P, D], BF16, tag="exp_m", bufs=10)
                                nc.vector.tensor_mul(
                                    eb, et, maskt_sb[:, kc, qc * D:(qc + 1) * D])
                                ex_tiles.append(eb)
                            for kc in range(8):
                                nc.tensor.matmul(pd, ones128b, ex_tiles[kc],
                                                 start=(kc == 0), stop=(kc == 7))
                            for kc in range(8):
                                nc.tensor.matmul(py, v_tm[:, b * 8 + kc, :],
                                                 ex_tiles[kc],
                                                 start=(kc == 0), stop=(kc == 7))
                            dr = ate.tile([1, D], F32, tag="dr", bufs=2)
                            nc.vector.reciprocal(dr, pd)
                            pb2 = psB.tile([HDIM, D], F32, tag="tr")
                            nc.tensor.matmul(pb2, ones1r[0:1, 0:HDIM], dr,
                                             start=True, stop=True)
                            db = ate.tile([HDIM, D], F32, tag="db", bufs=2)
                            nc.scalar.activation(db, pb2, AFT.Copy)
                            nc.vector.tensor_mul(yhT[:, qsl], py, db)
                    # proj partial -> DRAM
                    for tk in range(16):
                        ps = psA.tile([P, D], F32, tag="mm")
                        nc.tensor.matmul(ps, yhT[:, tk * P:(tk + 1) * P], wproj_sb,
                                         start=True, stop=True)
                        ev = ate.tile([P, D], BF16, tag="yp_ev")
                        nc.scalar.activation(ev, ps, AFT.Copy)
                        nc.sync.dma_start(ar1_in[tk * P:(tk + 1) * P, :], ev)

                    nc.gpsimd.collective_compute(
                        "AllReduce", mybir.AluOpType.add,
                        ins=[ar1_in[:]], outs=[ar1_out[:]], replica_groups=groups)

                # ---- x_res (DVE only); xrTb via xtm-transpose + yprojT tDMA ----
                with tc.tile_pool(name="s1t", bufs=4) as s1t:
                    # transpose x again (pre-collective data; PE idles during AR)
                    xT2 = [s1t.tile([P, N], BF16, tag="xT2", name=f"xT2{i}", bufs=4)
                           for i in range(4)]
                    for dc in range(4):
                        for g in range(16):
                            pt = psB.tile([P, P], F32, tag="tr")
                            nc.tensor.transpose(pt, xtm[:, g, dc * P:(dc + 1) * P],
                                                ident)
                            nc.scalar.activation(xT2[dc][:, g * P:(g + 1) * P],
                                                 pt, AFT.Copy)
                    yfull = s1t.tile([P, 16, D], BF16, tag="yfull", bufs=1)
                    nc.sync.dma_start(yfull,
                                      ar1_out[:, :].rearrange("(g p) d -> p g d", p=P))
                    projb_sb = s1t.tile([P, D], F32, tag="projb_sb", bufs=1)
                    nc.sync.dma_start(projb_sb, projb_d[:, :])
                    x_res = s1t.tile([P, 16, D], F32, tag="x_res", bufs=1)
                    for g in range(16):
                        t = s1t.tile([P, D], F32, tag="xr_t", bufs=3)
                        nc.vector.tensor_add(t, yfull[:, g, :], projb_sb)
                        nc.vector.tensor_add(x_res[:, g, :], t, xtm[:, g, :])
                    nc.sync.dma_start(
                        xres_dram[:, :].rearrange("(g p) d -> p g d", p=P), x_res)
                    # yproj^T via XBAR transpose-DMA (bf16), then xrTb on DVE
                    for dc in range(4):
                        ypT = s1t.tile([P, N], BF16, tag="ypT", bufs=2)
                        nc.sync.dma_start_transpose(
                            ypT, ar1_out[:, dc * P:(dc + 1) * P])
                        tb = s1t.tile([P, N], BF16, tag="tb", bufs=2)
                        nc.vector.tensor_scalar_add(tb, ypT,
                                                    projv_sb[:, dc:dc + 1])
                        nc.vector.tensor_add(xrTb[dc], tb, xT2[dc])
                    # router (bf16 matmul), softmax, top-8 gates
                    routes = s1t.tile([P, 16, E], F32, tag="routes", bufs=1)
                    for tk in range(16):
                        ps = psC.tile([P, E], F32, tag="ps_small")
                        for kc in range(4):
                            nc.tensor.matmul(ps, xrTb[kc][:, tk * P:(tk + 1) * P],
                                             rw_sb[:, kc, :],
                                             start=(kc == 0), stop=(kc == 3))
                        nc.vector.tensor_add(routes[:, tk, :], ps, rb_sb)
                    nc.scalar.activation(routes, routes, AFT.Exp)
                    rsum = s1t.tile([P, 16], F32, tag="rsum", bufs=1)
                    nc.vector.reduce_sum(rsum, routes, axis=mybir.AxisListType.X)
                    nc.vector.reciprocal(rsum, rsum)
                    gates = s1t.tile([P, 16, E], F32, tag="gates", bufs=1)
                    for g in range(16):
                        nc.vector.tensor_scalar_mul(routes[:, g, :], routes[:, g, :],
                                                    rsum[:, g:g + 1])
                        m8 = s1t.tile([P, 8], F32, tag="m8", bufs=2)
                        nc.vector.max(out=m8, in_=routes[:, g, :])
                        zap = s1t.tile([P, E], F32, tag="zap", bufs=2)
                        nc.vector.match_replace(out=zap, in_to_replace=m8,
                                                in_values=routes[:, g, :],
                                                imm_value=0)
                        nc.vector.tensor_sub(gates[:, g, :], routes[:, g, :], zap)
                    gsum = s1t.tile([P, 16], F32, tag="gsum", bufs=1)
                    nc.vector.reduce_sum(gsum, gates, axis=mybir.AxisListType.X)
                    nc.vector.reciprocal(gsum, gsum)
                    gates_bf = s1t.tile([P, 16, E], BF16, tag="gates_bf", bufs=1)
                    for g in range(16):
                        nc.vector.tensor_scalar_mul(gates[:, g, :], gates[:, g, :],
                                                    gsum[:, g:g + 1])
                        nc.vector.tensor_copy(gates_bf[:, g, :], gates[:, g, :])
                    nc.sync.dma_start(
                        gdram[:, :].rearrange("(g p) e -> p g e", p=P), gates_bf)
                    nc.sync.dma_start_transpose(gatesT, gdram[:, :])

            # ---- experts: dense eval of 2 local experts ----
            with tc.tile_pool(name="acts", bufs=17) as ac, \
                 tc.tile_pool(name="wst", bufs=2) as ws, \
                 tc.tile_pool(name="wca", bufs=5) as wc, \
                 tc.tile_pool(name="eev", bufs=3) as ev_:
                for e in range(EL):
                    win_t = [wc.tile([P, HD], BF16, tag="win", name=f"win{i}", bufs=5) for i in range(4)]
                    for kc in range(4):
                        nc.sync.dma_start(win_t[kc], win_d[e, kc * P:(kc + 1) * P, :])
                    wout_t = [wc.tile([P, 16, P], BF16, tag="wot", name=f"wot{i}", bufs=5) for i in range(4)]
                    for dc in range(4):
                        nc.sync.dma_start(
                            wout_t[dc], wout_d[e, :, dc * P:(dc + 1) * P]
                            .rearrange("(kc p) j -> p kc j", p=P))
                    for nb in range(NB):
                        tsl = slice(nb * TB, (nb + 1) * TB)
                        # h1
                        h1T = [ac.tile([P, TB], BF16, tag="h1T", name=f"h1T{i}") for i in range(16)]
                        for hc in range(16):
                            ps = psA.tile([P, TB], F32, tag="mm")
                            for kc in range(4):
                                nc.tensor.matmul(ps, win_t[kc][:, hc * P:(hc + 1) * P],
                                                 xrTb[kc][:, tsl],
                                                 start=(kc == 0), stop=(kc == 3))
                            nc.vector.tensor_scalar_add(h1T[hc], ps,
                                                        bin_sb[:, e, hc:hc + 1])
                        # c -> SwiGLU -> sT
                        sT = [ac.tile([P, TB], BF16, tag="sT", name=f"sT{i}") for i in range(16)]
                        for mc in range(16):
                            wa = ws.tile([P, 16, P], BF16, tag="w1a")
                            nc.sync.dma_start(
                                wa, w1_d[e, :, mc * P:(mc + 1) * P]
                                .rearrange("(kc p) j -> p kc j", p=P))
                            wb = ws.tile([P, 16, P], BF16, tag="w1b")
                            nc.sync.dma_start(
                                wb, w1_d[e, :, (mc + 16) * P:(mc + 17) * P]
                                .rearrange("(kc p) j -> p kc j", p=P))
                            pa = psA.tile([P, TB], F32, tag="mm")
                            pb = psA.tile([P, TB], F32, tag="mm")
                            for kc in range(16):
                                nc.tensor.matmul(pa, wa[:, kc, :], h1T[kc],
                                                 start=(kc == 0), stop=(kc == 15))
                            for kc in range(16):
                                nc.tensor.matmul(pb, wb[:, kc, :], h1T[kc],
                                                 start=(kc == 0), stop=(kc == 15))
                            sil = ev_.tile([P, TB], F32, tag="sil")
                            nc.scalar.activation(sil, pb, AFT.Silu,
                                                 bias=b1_sb[:, e, mc + 16:mc + 17])
                            av = ev_.tile([P, TB], F32, tag="av")
                            nc.vector.tensor_scalar_add(av, pa,
                                                        b1_sb[:, e, mc:mc + 1])
                            nc.vector.tensor_mul(sT[mc], sil, av)
                        # o
                        oT = [ac.tile([P, TB], BF16, tag="oT", name=f"oT{i}") for i in range(16)]
                        for oc in range(16):
                            w2t = ws.tile([P, 16, P], BF16, tag="w2t")
                            nc.sync.dma_start(
                                w2t, w2_d[e, :, oc * P:(oc + 1) * P]
                                .rearrange("(kc p) j -> p kc j", p=P))
                            ps = psA.tile([P, TB], F32, tag="mm")
                            for kc in range(16):
                                nc.tensor.matmul(ps, w2t[:, kc, :], sT[kc],
                                                 start=(kc == 0), stop=(kc == 15))
                            nc.vector.tensor_scalar_add(oT[oc], ps,
                                                        b2_sb[:, e, oc:oc + 1])
                        # eo + gate combine
                        pg = psB.tile([P, TB], F32, tag="tr")
                        nc.tensor.matmul(pg, sel_sb[:, e, :], gatesT[:, tsl],
                                         start=True, stop=True)
                        gb = ev_.tile([P, TB], F32, tag="gb")
                        nc.scalar.activation(gb, pg, AFT.Copy)
                        for dc in range(4):
                            ps = psA.tile([P, TB], F32, tag="mm")
                            for kc in range(16):
                                nc.tensor.matmul(ps, wout_t[dc][:, kc, :], oT[kc],
                                                 start=(kc == 0), stop=(kc == 15))
                            eo = ev_.tile([P, TB], F32, tag="eo")
                            nc.vector.tensor_scalar_add(eo, ps,
                                                        bout_sb[:, e, dc:dc + 1])
                            if e == 0:
                                nc.vector.tensor_mul(moeT[dc][:, tsl], eo, gb)
                            else:
                                t2 = ev_.tile([P, TB], F32, tag="t2")
                                nc.vector.tensor_mul(t2, eo, gb)
                                nc.vector.tensor_add(moeT[dc][:, tsl],
                                                     moeT[dc][:, tsl], t2)

            # ---- AllReduce moe; out = x_res + moe ----
            for dc in range(4):
                nc.sync.dma_start(ar2_in[dc * P:(dc + 1) * P, :], moeT[dc])
            nc.gpsimd.collective_compute(
                "AllReduce", mybir.AluOpType.add,
                ins=[ar2_in[:]], outs=[ar2_out[:]], replica_groups=groups)
            with tc.tile_pool(name="fin", bufs=2) as fi:
                xres_tm = fi.tile([P, 16, D], F32, tag="xres_tm", bufs=1)
                nc.sync.dma_start(xres_tm,
                                  xres_dram[:, :].rearrange("(g p) d -> p g d", p=P))
                for g in range(16):
                    mt = fi.tile([P, D], BF16, tag="mt", bufs=3)
                    nc.sync.dma_start_transpose(
                        mt, ar2_out[:, g * P:(g + 1) * P])
                    ov = fi.tile([P, D], F32, tag="o_ev", bufs=3)
                    nc.vector.tensor_add(ov, mt, xres_tm[:, g, :])
                    nc.sync.dma_start(out_d[g * P:(g + 1) * P, :], ov)

    _split_matmul_waits(nc)
    return nc


def _split_matmul_waits(nc):
    """walrus allows only one sync-wait per engine-instruction sync slot; move
    extra waits onto standalone InstEventSemaphore waits inserted before."""
    import concourse.mybir as mybir
    k = 0
    for bb in nc.main_func.blocks:
        il = list(bb.instructions)
        out = []
        changed = False
        for ins in il:
            si = getattr(ins, "sync_info", None)
            if si is not None and len(si.on_wait) > 1 \
                    and type(ins).__name__ != "InstEventSemaphore":
                waits = list(si.on_wait)
                keep, move = waits[-1], waits[:-1]
                for w in move:
                    nop = mybir.InstEventSemaphore(name=f"I-wsplit-{k}",
                                                   ins=[], outs=[])
                    k += 1
                    nop.engine = ins.engine
                    nop.sync_info = type(si)(on_wait=[w], on_update=[])
                    out.append(nop)
                ins.sync_info = type(si)(on_wait=[keep],
                                         on_update=list(si.on_update))
                changed = True
            out.append(ins)
        if changed:
            bb.instructions = out


def _prep_inputs(inputs, core):
    bf = ml_dtypes.bfloat16
    f32 = np.float32
    h = core
    sl = slice(2 * core, 2 * core + 2)
    caw = np.asarray(inputs["c_attn_w"], f32)
    cab = np.asarray(inputs["c_attn_b"], f32)
    wqkv = np.concatenate([
        caw[:, h * 64:(h + 1) * 64],
        caw[:, 512 + h * 64:512 + (h + 1) * 64],
        caw[:, 1024 + h * 64:1024 + (h + 1) * 64]], axis=1)
    bqkv = np.stack([
        cab[h * 64:(h + 1) * 64],
        cab[512 + h * 64:512 + (h + 1) * 64],
        cab[1024 + h * 64:1024 + (h + 1) * 64]]).astype(f32)
    selb = np.zeros((EL, E, P), bf)
    selb[0, 2 * core, :] = 1.0
    selb[1, 2 * core + 1, :] = 1.0
    return {
        "x": np.asarray(inputs["x"], f32).reshape(N, D),
        "gvec": np.asarray(inputs["g"], f32).reshape(4, P),
        "bvec": np.asarray(inputs["b"], f32).reshape(4, P),
        "wqkv": wqkv.astype(bf),
        "bqkv": bqkv,
        "alpha_s": np.asarray(inputs["alpha"], f32)[h].reshape(1, 1),
        "maskt": np.triu(np.ones((T, T), f32)).astype(bf),
        "wproj": np.asarray(inputs["c_proj_w"], f32)[h * 64:(h + 1) * 64, :].astype(bf),
        "projb_bc": np.broadcast_to(np.asarray(inputs["c_proj_b"], f32),
                                    (P, D)).copy(),
        "projb_vec": np.asarray(inputs["c_proj_b"], f32).reshape(4, P),
        "vbias_bc": np.broadcast_to(bqkv[2], (P, HDIM)).copy(),
        "rw": np.asarray(inputs["router_w"], f32).astype(bf),
        "rb_bc": np.broadcast_to(np.asarray(inputs["router_b"], f32), (P, E)).copy(),
        "selb": selb,
        "w_in_l": np.asarray(inputs["w_in"], f32)[sl].astype(bf),
        "b_in_l": np.asarray(inputs["b_in"], f32)[sl].reshape(EL, 16, P),
        "w1_l": np.asarray(inputs["w1"], f32)[sl].astype(bf),
        "b1_l": np.asarray(inputs["b1"], f32)[sl].reshape(EL, 32, P),
        "w2_l": np.asarray(inputs["w2"], f32)[sl].astype(bf),
        "b2_l": np.asarray(inputs["b2"], f32)[sl].reshape(EL, 16, P),
        "w_out_l": np.asarray(inputs["w_out"], f32)[sl].astype(bf),
        "b_out_l": np.asarray(inputs["b_out"], f32)[sl].reshape(EL, 4, P),
    }


def kernel(**inputs):
    if "nc" not in _cache:
        _cache["nc"] = build_program()
    nc = _cache["nc"]
    in_maps = [_prep_inputs(inputs, c) for c in range(NCORES)]
    res = run_bass_kernel_spmd(nc, in_maps, core_ids=list(range(NCORES)))
    out = res.results[0]["out"]
    return np.asarray(out, np.float32).reshape(2, 1024, 512)
